# revision 17
# baseline (speedup 1.0000x reference)
"""Guided attention kernel for Trainium2, 8-core data-parallel over batch.

Math per batch b (C=64, D=8, N=H*W=4096):
  q = Wq @ query + bq            [D, N]
  k = Wk @ query + bk            [D, N]
  v = Wv @ value + bv            [C, N]
  E[n, m] = sum_d q[d, n] k[d, m]
  A = softmax_m(E)
  out[c, n] = sum_m v[c, m] A[n, m] + value[c, n]

Device strategy (one batch per NeuronCore):
  - Host augments inputs: xq = [query; 1] (65, N), xv = [value; 1] (65, N),
    gt = wq_aug @ wk_aug^T (65, 65) where wq_aug = [Wq^T; bq] (65, 8),
    wv = [[Wv^T, 0]; [bv, 1]] (65, 128) -> vt[m, 64] == 1 (ones column
    makes the output matmul also produce softmax row sums for free).
  - The q/k projections are folded into the energy matmul:
      E^T = xq^T (wk_aug wq_aug^T) xq = xq^T kg,  kg := gt^T xq  [65, N]
    so every energy matmul contracts over 65 partitions instead of 8
    (K=8 matmuls run ~2x slower per row on the PE due to LDWEIGHTS cost).
  - Energy computed transposed, E^T[m, n] (PE), exp on ScalarE directly
    from PSUM (no row-max subtraction: |E| < 30, safe in fp32), expET
    feeds the output matmul as the moving operand, so no transpose of the
    big [N, N] matrix is ever needed.
  - o_ps[c', n] = sum_m vt[m, c'] expET[m, n]; row 64 is the softmax
    denominator. Epilogue: hop the denominator to partition 0 (64-aligned
    DVE shift), reciprocal_approx_fast, then a rank-1 (K=1) PE matmul
    broadcasts the reciprocal row across partitions 0..63 in PSUM.
  - Input DMAs are chunked [65, 1024] and issued from both SP and ACT
    HWDGE paths so projections (vt, kg) start while later chunks stream.
  - Main loop: 2-m-chunk rounds, triple-buffered energy PSUM (3x2 banks)
    + double-buffered out accumulators (2 banks), so the PE can run two
    energy rounds ahead of the exp consumer.
"""

import sys

sys.path.insert(0, "/opt/trn_rl_repo")

import numpy as np

import concourse.bacc as bacc
import concourse.bass as bass
import concourse.tile as tile
from concourse import mybir
from concourse.bass_utils import run_bass_kernel_spmd

F32 = mybir.dt.float32
F32R = mybir.dt.float32r
EXP = mybir.ActivationFunctionType.Exp

C = 64
CH = 65             # augmented channels (64 + ones row)
N = 4096
NG = 512            # n-group width (columns per psum bank)
NGROUPS = N // NG   # 8
MC = 128            # m-chunk width
MCHUNKS = N // MC   # 32
XCH = 1024          # input dma chunk width
RSZ = 2             # m-chunks per exp round
MGROUPS = [RSZ] * (MCHUNKS // RSZ)

TRACE = False
_CACHE = {}


def build_program():
    nc = bacc.Bacc("TRN2", debug=False)

    xq_d = nc.dram_tensor("xq", [CH, N], F32R, kind="ExternalInput")
    xv_d = nc.dram_tensor("xv", [CH, N], F32R, kind="ExternalInput")
    gt_d = nc.dram_tensor("gt", [CH, CH], F32R, kind="ExternalInput")
    wv_d = nc.dram_tensor("wv", [CH, MC], F32R, kind="ExternalInput")
    ones_d = nc.dram_tensor("ones", [1, C], F32R, kind="ExternalInput")
    out_d = nc.dram_tensor("out", [C, N], F32, kind="ExternalOutput")

    with (
        tile.TileContext(nc) as tc,
        tc.tile_pool(name="consts", bufs=1) as consts,
        tc.tile_pool(name="expp", bufs=3) as expp,
        tc.tile_pool(name="small", bufs=2) as small,
        tc.tile_pool(name="pe_ps", bufs=3, space="PSUM") as pe_ps,
        tc.tile_pool(name="po_ps", bufs=2, space="PSUM") as po_ps,
    ):
        xq_sb = consts.tile([CH, N], F32R)
        xv_sb = consts.tile([CH, N], F32R)
        kg_sb = consts.tile([CH, N], F32R)
        vt_sb = consts.tile([MC, N], F32R)
        out_sb = consts.tile([C, N], F32)
        gt_sb = consts.tile([CH, CH], F32R)
        wv_sb = consts.tile([CH, MC], F32R)
        ones_sb = consts.tile([1, C], F32R)

        nc.sync.dma_start(out=wv_sb, in_=wv_d[:])
        nc.scalar.dma_start(out=gt_sb, in_=gt_d[:])
        nc.scalar.dma_start(out=ones_sb, in_=ones_d[:])
        nxch = N // XCH
        for j in range(nxch):
            cols = slice(j * XCH, (j + 1) * XCH)
            nc.sync.dma_start(out=xv_sb[:, cols], in_=xv_d[:, cols])
            nc.scalar.dma_start(out=xq_sb[:, cols], in_=xq_d[:, cols])

        # --- head: vt projection + kg, interleaved per input chunk; vt
        # first so the PE starts as soon as xv chunk 0 lands ---
        for j in range(nxch):
            for mi in range(j * MCHUNKS // nxch, (j + 1) * MCHUNKS // nxch):
                mcols = slice(mi * MC, (mi + 1) * MC)
                ps_vt = po_ps.tile([MC, MC], F32, tag="o", name=f"ps_vt{mi}")
                nc.tensor.matmul(out=ps_vt[:], lhsT=xv_sb[:, mcols], rhs=wv_sb[:])
                nc.vector.tensor_copy(vt_sb[:, mcols], ps_vt[:])
            for g in range(j * NGROUPS // nxch, (j + 1) * NGROUPS // nxch):
                ncols = slice(g * NG, (g + 1) * NG)
                ps_kg = po_ps.tile([CH, NG], F32, tag="o", name=f"ps_kg{g}")
                nc.tensor.matmul(out=ps_kg[:], lhsT=gt_sb[:], rhs=xq_sb[:, ncols])
                nc.vector.tensor_copy(kg_sb[:, ncols], ps_kg[:])

        # --- main attention loop, software-pipelined by one exp round so
        # the PE never sits behind an ACT wait in its own program order ---
        rounds = []
        for g in range(NGROUPS):
            mi = 0
            for msz in MGROUPS:
                rounds.append((g, mi, msz))
                mi += msz

        o_tiles = {}

        def emit_out_round(g, mi, msz, ex):
            if g not in o_tiles:
                o_tiles[g] = po_ps.tile([MC, NG], F32, tag="o", name=f"o_ps{g}")
            o_ps = o_tiles[g]
            for j in range(msz):
                vcols = slice((mi + j) * MC, (mi + j) * MC + CH)
                nc.tensor.matmul(
                    out=o_ps[0:CH, :],
                    lhsT=vt_sb[:, vcols],
                    rhs=ex[:, j * NG:(j + 1) * NG],
                    start=(mi + j == 0),
                    stop=(mi + j == MCHUNKS - 1),
                )
            if mi + msz == MCHUNKS:
                emit_epilogue(g, o_ps)

        def emit_epilogue(g, o_ps):
            # o_ps rows 0..63 = channels, row 64 = softmax denominator.
            # reciprocal_approx_fast requires base partition 0, so hop the
            # denominator row 64 -> 0 first (64-aligned DVE shifts are legal).
            ncols = slice(g * NG, (g + 1) * NG)
            den0 = small.tile([1, NG], F32, tag="den", name=f"den{g}")
            nc.vector.tensor_copy(den0[:], o_ps[C:C + 1, :])
            rec = small.tile([1, NG], F32R, tag="rec", name=f"rec{g}")
            nc.vector.reciprocal_approx_fast(out=rec[:].bitcast(F32), in_=den0[:])
            rec_r = small.tile([1, NG], F32R, tag="recr", name=f"recr{g}")
            nc.vector.tensor_copy(rec_r[:], rec[:].bitcast(F32))
            # rank-1 PE matmul broadcasts the reciprocal row to 64 partitions
            bc_ps = pe_ps.tile([C, NG], F32, tag="e", name=f"bc{g}")
            nc.tensor.matmul(out=bc_ps[:], lhsT=ones_sb[:], rhs=rec_r[:])
            rec_bc = small.tile([C, NG], F32, tag="recb", name=f"recb{g}")
            nc.vector.tensor_copy(rec_bc[:], bc_ps[:])
            nc.vector.tensor_mul(out_sb[:, ncols], o_ps[0:C, :], rec_bc[:])
            nc.vector.tensor_add(out_sb[:, ncols], out_sb[:, ncols],
                                 xv_sb[0:C, ncols].bitcast(F32))
            nc.sync.dma_start(out=out_d[:, ncols], in_=out_sb[:, ncols])

        pending = None
        for ridx, (g, mi, msz) in enumerate(rounds):
            ncols = slice(g * NG, (g + 1) * NG)
            w = msz * NG
            e_ps = pe_ps.tile([MC, RSZ * NG], F32, tag="e", name=f"e_ps{ridx}")
            ex = expp.tile([MC, RSZ * NG], F32R, tag="ex", name=f"ex{ridx}")
            for j in range(msz):
                mcols = slice((mi + j) * MC, (mi + j + 1) * MC)
                nc.tensor.matmul(
                    out=e_ps[:, j * NG:(j + 1) * NG],
                    lhsT=xq_sb[:, mcols],
                    rhs=kg_sb[:, ncols],
                )
            nc.scalar.activation(out=ex[:, :w], in_=e_ps[:, :w], func=EXP)
            if pending is not None:
                emit_out_round(*pending)
            pending = (g, mi, msz, ex)
        emit_out_round(*pending)

    nc.finalize()
    return nc


def get_program():
    if "nc" not in _CACHE:
        _CACHE["nc"] = build_program()
    return _CACHE["nc"]


def prep_inputs(query, value, Wq, bq, Wk, bk, Wv, bv):
    B = query.shape[0]
    ones = np.ones((B, 1, N), np.float32)
    xq = np.concatenate([query.reshape(B, C, N).astype(np.float32), ones], axis=1)
    xv = np.concatenate([value.reshape(B, C, N).astype(np.float32), ones], axis=1)
    wq_aug = np.concatenate([Wq.T, bq[None, :]], axis=0).astype(np.float64)
    wk_aug = np.concatenate([Wk.T, bk[None, :]], axis=0).astype(np.float64)
    gt = (wq_aug @ wk_aug.T).astype(np.float32)
    wv_ = np.zeros((CH, MC), np.float32)
    wv_[:C, :C] = Wv.T
    wv_[C, :C] = bv
    wv_[C, C] = 1.0
    ones_row = np.ones((1, C), np.float32)
    return [
        {
            "xq": np.ascontiguousarray(xq[b]),
            "xv": np.ascontiguousarray(xv[b]),
            "gt": gt,
            "wv": wv_,
            "ones": ones_row,
        }
        for b in range(B)
    ]


def kernel(query, value, Wq, bq, Wk, bk, Wv, bv):
    query = np.asarray(query)
    value = np.asarray(value)
    B, _, H, W = query.shape
    in_maps = prep_inputs(
        query, value,
        np.asarray(Wq), np.asarray(bq), np.asarray(Wk),
        np.asarray(bk), np.asarray(Wv), np.asarray(bv),
    )
    nc = get_program()
    try:
        res = run_bass_kernel_spmd(nc, in_maps, core_ids=list(range(B)), trace=TRACE)
    except ModuleNotFoundError:
        res = run_bass_kernel_spmd(nc, in_maps, core_ids=list(range(B)), trace=False)
    _CACHE["last_result"] = res
    out = np.stack([res.results[b]["out"] for b in range(B)])
    return out.reshape(B, C, H, W).astype(query.dtype)


# revision 23
# speedup vs baseline: 1.1085x; 1.1085x over previous
"""Guided attention kernel for Trainium2, 8-core data-parallel over batch.

Math per batch b (C=64, D=8, N=H*W=4096):
  q = Wq @ query + bq            [D, N]
  k = Wk @ query + bk            [D, N]
  v = Wv @ value + bv            [C, N]
  E[n, m] = sum_d q[d, n] k[d, m]
  A = softmax_m(E)
  out[c, n] = sum_m v[c, m] A[n, m] + value[c, n]

Device strategy (one batch per NeuronCore):
  - Host augments inputs: xq = [query; 1] (65, N), xv = [value; 1] (65, N),
    gt = wq_aug @ wk_aug^T (65, 65) where wq_aug = [Wq^T; bq] (65, 8),
    wv = [[Wv^T, 0]; [bv, 1]] (65, 128) -> vt[m, 64] == 1 (ones column
    makes the output matmul also produce softmax row sums for free).
  - The q/k projections are folded into the energy matmul:
      E^T = xq^T (wk_aug wq_aug^T) xq = xq^T kg,  kg := gt^T xq  [65, N]
    so every energy matmul contracts over 65 partitions instead of 8
    (K=8 matmuls run ~2x slower per row on the PE due to LDWEIGHTS cost).
  - Energy computed transposed, E^T[m, n] (PE), exp on ScalarE directly
    from PSUM (no row-max subtraction: |E| < 30, safe in fp32), expET
    feeds the output matmul as the moving operand, so no transpose of the
    big [N, N] matrix is ever needed.
  - o_ps[c', n] = sum_m vt[m, c'] expET[m, n]; row 64 is the softmax
    denominator. Epilogue: hop the denominator to partition 0 (64-aligned
    DVE shift), reciprocal_approx_fast, then a rank-1 (K=1) PE matmul
    broadcasts the reciprocal row across partitions 0..63 in PSUM.
  - Input DMAs are chunked [65, 1024] and issued from both SP and ACT
    HWDGE paths so projections (vt, kg) start while later chunks stream.
  - Main loop: 2-m-chunk rounds, triple-buffered energy PSUM (3x2 banks)
    + double-buffered out accumulators (2 banks), so the PE can run two
    energy rounds ahead of the exp consumer.
"""

import sys

sys.path.insert(0, "/opt/trn_rl_repo")

import numpy as np

import concourse.bacc as bacc
import concourse.bass as bass
import concourse.tile as tile
from concourse import mybir
from concourse.bass_utils import run_bass_kernel_spmd

F32 = mybir.dt.float32
F32R = mybir.dt.float32r
EXP = mybir.ActivationFunctionType.Exp

C = 64
CH = 65             # augmented channels (64 + ones row)
N = 4096
NG = 512            # n-group width (columns per psum bank)
NGROUPS = N // NG   # 8
MC = 128            # m-chunk width
MCHUNKS = N // MC   # 32
XCH = 1024          # input dma chunk width
# m-chunks per exp round: 3 banks of energy psum per round (3+3 ping-pong
# + 2 banks for the out accumulator = 8 banks total)
MGROUPS = [3] * 10 + [2]
RSZ = 3

TRACE = False
_CACHE = {}


def build_program():
    nc = bacc.Bacc("TRN2", debug=False)

    xq_d = nc.dram_tensor("xq", [CH, N], F32R, kind="ExternalInput")
    xv_d = nc.dram_tensor("xv", [CH, N], F32R, kind="ExternalInput")
    gt_d = nc.dram_tensor("gt", [CH, CH], F32R, kind="ExternalInput")
    wv_d = nc.dram_tensor("wv", [CH, MC], F32R, kind="ExternalInput")
    out_d = nc.dram_tensor("out", [C, N], F32, kind="ExternalOutput")
    rec_d = nc.dram_tensor("recscratch", [NGROUPS, NG], F32, kind="Internal")

    with (
        tile.TileContext(nc) as tc,
        tc.tile_pool(name="consts", bufs=1) as consts,
        tc.tile_pool(name="expp", bufs=3) as expp,
        tc.tile_pool(name="small", bufs=2) as small,
        tc.tile_pool(name="pe_ps", bufs=2, space="PSUM") as pe_ps,
        tc.tile_pool(name="po_ps", bufs=2, space="PSUM") as po_ps,
    ):
        xq_sb = consts.tile([CH, N], F32R)
        xv_sb = consts.tile([CH, N], F32R)
        kg_sb = consts.tile([CH, N], F32R)
        vt_sb = consts.tile([MC, N], F32R)
        out_sb = consts.tile([C, N], F32)
        gt_sb = consts.tile([CH, CH], F32R)
        wv_sb = consts.tile([CH, MC], F32R)

        nc.sync.dma_start(out=wv_sb, in_=wv_d[:])
        nc.scalar.dma_start(out=gt_sb, in_=gt_d[:])
        nxch = N // XCH
        for j in range(nxch):
            cols = slice(j * XCH, (j + 1) * XCH)
            nc.sync.dma_start(out=xv_sb[:, cols], in_=xv_d[:, cols])
            nc.scalar.dma_start(out=xq_sb[:, cols], in_=xq_d[:, cols])

        # --- head: vt projection + kg, interleaved per input chunk; vt
        # first so the PE starts as soon as xv chunk 0 lands ---
        for j in range(nxch):
            for mi in range(j * MCHUNKS // nxch, (j + 1) * MCHUNKS // nxch):
                mcols = slice(mi * MC, (mi + 1) * MC)
                ps_vt = po_ps.tile([MC, MC], F32, tag="o", name=f"ps_vt{mi}")
                nc.tensor.matmul(out=ps_vt[:], lhsT=xv_sb[:, mcols], rhs=wv_sb[:])
                nc.vector.tensor_copy(vt_sb[:, mcols], ps_vt[:])
            for g in range(j * NGROUPS // nxch, (j + 1) * NGROUPS // nxch):
                ncols = slice(g * NG, (g + 1) * NG)
                ps_kg = po_ps.tile([CH, NG], F32, tag="o", name=f"ps_kg{g}")
                nc.tensor.matmul(out=ps_kg[:], lhsT=gt_sb[:], rhs=xq_sb[:, ncols])
                nc.vector.tensor_copy(kg_sb[:, ncols], ps_kg[:])

        # --- main attention loop, software-pipelined by one exp round so
        # the PE never sits behind an ACT wait in its own program order ---
        rounds = []
        for g in range(NGROUPS):
            mi = 0
            for msz in MGROUPS:
                rounds.append((g, mi, msz))
                mi += msz

        o_tiles = {}

        def emit_out_round(g, mi, msz, ex):
            if g not in o_tiles:
                o_tiles[g] = po_ps.tile([MC, NG], F32, tag="o", name=f"o_ps{g}")
            o_ps = o_tiles[g]
            for j in range(msz):
                vcols = slice((mi + j) * MC, (mi + j) * MC + CH)
                nc.tensor.matmul(
                    out=o_ps[0:CH, :],
                    lhsT=vt_sb[:, vcols],
                    rhs=ex[:, j * NG:(j + 1) * NG],
                    start=(mi + j == 0),
                    stop=(mi + j == MCHUNKS - 1),
                )
            if mi + msz == MCHUNKS:
                emit_epilogue(g, o_ps)

        def emit_epilogue(g, o_ps):
            # o_ps rows 0..63 = channels, row 64 = softmax denominator.
            # reciprocal_approx_fast requires base partition 0, so hop the
            # denominator row 64 -> 0 first (64-aligned DVE shifts are legal).
            ncols = slice(g * NG, (g + 1) * NG)
            den0 = small.tile([1, NG], F32, tag="den", name=f"den{g}")
            nc.vector.tensor_copy(den0[:], o_ps[C:C + 1, :])
            rec = small.tile([1, NG], F32, tag="rec", name=f"rec{g}")
            nc.vector.reciprocal_approx_fast(out=rec[:], in_=den0[:])
            # broadcast the reciprocal row across 64 partitions via a DRAM
            # bounce (partition-stride-0 read); latency hides in the pipeline
            nc.sync.dma_start(out=rec_d[g:g + 1, :], in_=rec[:])
            rd = rec_d[g:g + 1, :]
            rec_bcast = bass.AP(tensor=rd.tensor, offset=rd.offset,
                                ap=[[0, C]] + list(rd.ap[1:]))
            rec_bc = small.tile([C, NG], F32, tag="recb", name=f"recb{g}")
            nc.sync.dma_start(out=rec_bc[:], in_=rec_bcast)
            nc.vector.tensor_mul(out_sb[:, ncols], o_ps[0:C, :], rec_bc[:])
            nc.vector.tensor_add(out_sb[:, ncols], out_sb[:, ncols],
                                 xv_sb[0:C, ncols].bitcast(F32))
            nc.sync.dma_start(out=out_d[:, ncols], in_=out_sb[:, ncols])

        pending = None
        for ridx, (g, mi, msz) in enumerate(rounds):
            ncols = slice(g * NG, (g + 1) * NG)
            w = msz * NG
            e_ps = pe_ps.tile([MC, RSZ * NG], F32, tag="e", name=f"e_ps{ridx}")
            ex = expp.tile([MC, RSZ * NG], F32R, tag="ex", name=f"ex{ridx}")
            for j in range(msz):
                mcols = slice((mi + j) * MC, (mi + j + 1) * MC)
                nc.tensor.matmul(
                    out=e_ps[:, j * NG:(j + 1) * NG],
                    lhsT=xq_sb[:, mcols],
                    rhs=kg_sb[:, ncols],
                )
            nc.scalar.activation(out=ex[:, :w], in_=e_ps[:, :w], func=EXP)
            if pending is not None:
                emit_out_round(*pending)
            pending = (g, mi, msz, ex)
        emit_out_round(*pending)

    nc.finalize()
    return nc


def get_program():
    if "nc" not in _CACHE:
        _CACHE["nc"] = build_program()
    return _CACHE["nc"]


def prep_inputs(query, value, Wq, bq, Wk, bk, Wv, bv):
    B = query.shape[0]
    ones = np.ones((B, 1, N), np.float32)
    xq = np.concatenate([query.reshape(B, C, N).astype(np.float32), ones], axis=1)
    xv = np.concatenate([value.reshape(B, C, N).astype(np.float32), ones], axis=1)
    wq_aug = np.concatenate([Wq.T, bq[None, :]], axis=0).astype(np.float64)
    wk_aug = np.concatenate([Wk.T, bk[None, :]], axis=0).astype(np.float64)
    gt = (wq_aug @ wk_aug.T).astype(np.float32)
    wv_ = np.zeros((CH, MC), np.float32)
    wv_[:C, :C] = Wv.T
    wv_[C, :C] = bv
    wv_[C, C] = 1.0
    return [
        {
            "xq": np.ascontiguousarray(xq[b]),
            "xv": np.ascontiguousarray(xv[b]),
            "gt": gt,
            "wv": wv_,
        }
        for b in range(B)
    ]


def kernel(query, value, Wq, bq, Wk, bk, Wv, bv):
    query = np.asarray(query)
    value = np.asarray(value)
    B, _, H, W = query.shape
    in_maps = prep_inputs(
        query, value,
        np.asarray(Wq), np.asarray(bq), np.asarray(Wk),
        np.asarray(bk), np.asarray(Wv), np.asarray(bv),
    )
    nc = get_program()
    try:
        res = run_bass_kernel_spmd(nc, in_maps, core_ids=list(range(B)), trace=TRACE)
    except ModuleNotFoundError:
        res = run_bass_kernel_spmd(nc, in_maps, core_ids=list(range(B)), trace=False)
    _CACHE["last_result"] = res
    out = np.stack([res.results[b]["out"] for b in range(B)])
    return out.reshape(B, C, H, W).astype(query.dtype)
